# revision 14
# baseline (speedup 1.0000x reference)
"""Trainium2 Bass kernel for CartNN minimal-NEAT forward pass.

Computes out = tanh(tanh(x @ w + b))[:, None] for x [16384, 4096] f32,
w [4096] f32, b [1] f32, data-parallel across 8 NeuronCores (2048 batch
rows per core). Memory-bound: each core streams its 32 MiB x shard once.

Per-core structure (measured on HW, iterated via NTFF profiles):
  - x streams as 16 [128, 4096] tiles on the sync HWDGE ring, which
    carries NOTHING else except the single leading 16 KiB w load: the 16
    HW queues run 16 KiB descriptors back-to-back at ~432 GB/s
    aggregate. Moving w/b onto the scalar ring (descriptors interleave
    mid-stream) left queue 15 running stretched ~745 ns descriptors for
    the rest of the kernel (-6 us); splitting tile 0's load into 4 KiB
    descriptors slowed the whole early stream to ~700-750 ns/descriptor
    for 40 us (-6 us). Keep the stream maximally regular.
  - w is broadcast to all 128 partitions by TensorE outer products
    ones[128,1] @ w[1,512] -> PSUM, copied PSUM->SBUF by ScalarE
    (~1.7 us/chunk). Copying on VectorE instead made EVERY subsequent
    DVE op 1.20x slower (4.42 -> 5.31 us/tile) - keep ScalarE.
  - The dot product is one fused mul+reduce VectorE op per tile
    (affine_mul_reduce, 4.42 us/tile vs 4.97 us/tile DMA delivery; the
    rate is ALU-issue-bound: a bf16 product output left the op at 4.42
    but shifted SBUF layout and cost 13 us elsewhere - keep all-f32).
    The first 4 tiles are split into quarter-K ops with a staggered
    emission (quarter q of tile t at step t + 3q) so DVE starts as soon
    as the first w chunks are broadcast. NO GpSimd/ScalarE offload: a
    GpSimd tensor_mul of one tile takes ~10 us and its SBUF traffic
    slowed a concurrent DVE op 4.4 -> 12.9 us.
  - Tiles 14/15 are split (loads AND compute: quarters for t=14, eighths
    for t=15) so the final DVE piece starts on the last 256 KiB of x;
    partials fold with one reduce_sum per tile.
  - Output: tanh(tanh(.+b)) on ScalarE, TensorE transpose [128,16] ->
    [16,128], one 8 KiB DMA of 512B-contiguous rows (the partition-major
    scatter cost a 12 us completion wait: 2048 4-byte descriptors).
"""

import numpy as np

import concourse.bacc as bacc
import concourse.mybir as mybir
from concourse.bass_utils import run_bass_kernel_spmd
from concourse.masks import make_identity
from concourse.tile import TileContext

N_CORES = 8
BATCH = 16384
IN_SIZE = 4096
P = 128
B_PER_CORE = BATCH // N_CORES  # 2048
N_TILES = B_PER_CORE // P  # 16

_NC_CACHE = None


def _build():
    nc = bacc.Bacc(
        "TRN2",
        target_bir_lowering=False,
        debug=False,
        num_devices=N_CORES,
    )
    x = nc.dram_tensor(
        "x", [B_PER_CORE, IN_SIZE], mybir.dt.float32, kind="ExternalInput"
    )
    w = nc.dram_tensor("w", [IN_SIZE], mybir.dt.float32, kind="ExternalInput")
    b = nc.dram_tensor("b", [1], mybir.dt.float32, kind="ExternalInput")
    y = nc.dram_tensor("y", [B_PER_CORE, 1], mybir.dt.float32, kind="ExternalOutput")

    xt = x.rearrange("(t p) k -> t p k", p=P)  # [16, 128, 4096]
    yT = y.rearrange("(t p) o -> t (p o)", p=P)  # [16, 128], 512B rows

    with TileContext(nc) as tc:
        with (
            tc.tile_pool(name="xpool", bufs=8) as xpool,
            tc.tile_pool(name="scratch", bufs=1) as spool,
            tc.tile_pool(name="consts", bufs=1) as cpool,
            tc.tile_pool(name="psum", bufs=1, space="PSUM") as ppool,
        ):
            # w first on the sync ring (single 16 KiB descriptor), then
            # the x stream owns the ring. b rides the scalar ring.
            w_1K = cpool.tile([1, IN_SIZE], mybir.dt.float32)
            nc.sync.dma_start(out=w_1K[:], in_=w[None, :])
            b_11 = cpool.tile([1, 1], mybir.dt.float32)
            nc.scalar.dma_start(out=b_11[:], in_=b[None, :])
            ones_1P = cpool.tile([1, P], mybir.dt.float32)
            nc.vector.memset(ones_1P[:], 1.0)
            w_PK = cpool.tile([P, IN_SIZE], mybir.dt.float32)
            NCHUNK = 512
            for c in range(IN_SIZE // NCHUNK):
                cs = slice(c * NCHUNK, (c + 1) * NCHUNK)
                w_psum = ppool.tile([P, NCHUNK], mybir.dt.float32, bufs=4)
                nc.tensor.matmul(w_psum[:], ones_1P[:], w_1K[0:1, cs])
                nc.scalar.copy(w_PK[:, cs], w_psum[:])
            b_psum = ppool.tile([P, 1], mybir.dt.float32)
            nc.tensor.matmul(b_psum[:], ones_1P[:], b_11[:])
            b_P1 = cpool.tile([P, 1], mybir.dt.float32)
            nc.scalar.copy(b_P1[:], b_psum[:])
            ident = cpool.tile([P, P], mybir.dt.float32)
            make_identity(nc, ident[:])

            # VectorE does one fused mul+reduce pass per tile. The first 4
            # tiles are split into quarter-K ops: quarter q only needs
            # w[q*1024:(q+1)*1024], so DVE starts as soon as the first w
            # chunks are broadcast instead of waiting for all of w. Later
            # tiles use a single full-K op. The Tile scheduler keeps
            # same-engine program order, so the order must be explicit.
            NSPLIT = 4
            NQT = 4  # tiles that use the quarter-split
            KQ = IN_SIZE // NSPLIT
            acc_PT = cpool.tile([P, N_TILES], mybir.dt.float32)
            accs_q = [
                cpool.tile([P, NQT], mybir.dt.float32, name=f"acc_{q}")
                for q in range(1, NSPLIT)
            ]
            prod_PK = spool.tile([P, IN_SIZE], mybir.dt.float32)
            x_tiles = {}

            def load_x(t):
                x_PK = xpool.tile([P, IN_SIZE], mybir.dt.float32)
                nc.sync.dma_start(out=x_PK[:], in_=xt[t])
                x_tiles[t] = x_PK

            def emit_quarter(t, q):
                seg = slice(q * KQ, (q + 1) * KQ)
                acc = acc_PT[:, t : t + 1] if q == 0 else accs_q[q - 1][:, t : t + 1]
                nc.vector.affine_mul_reduce(
                    out=prod_PK[:, seg],
                    accum_out=acc,
                    in0=x_tiles[t][:, seg],
                    in1=w_PK[:, seg],
                    scale=1.0,
                    bias=0.0,
                )

            # Tile 0's load is split into quarters so DVE's first op needs
            # only the first 256 KiB of x plus w chunks 0-1 (measured DVE
            # start 15.4 vs 19.4 us on a cold draw). name="x_PK" keeps it
            # in the same pool slot group as the other x tiles (slot
            # groups key on the tile name; a distinct name doubles the
            # pool and overflows SBUF).
            x0 = xpool.tile([P, IN_SIZE], mybir.dt.float32, name="x_PK")
            x_tiles[0] = x0
            for s in range(NSPLIT):
                seg = slice(s * KQ, (s + 1) * KQ)
                nc.sync.dma_start(out=x0[:, seg], in_=xt[0][:, seg])
            for t in range(1, NQT):
                load_x(t)
            # Greedy emission order: each (t, q) placed by max(w-chunk
            # ready ~11+1.7(2q+1), x-tile land) so the in-order DVE never
            # waits for a w chunk that hasn't been broadcast yet.
            ORDER = [
                (0, 0), (0, 1), (1, 0), (1, 1), (0, 2), (2, 0), (1, 2),
                (0, 3), (2, 1), (3, 0), (1, 3), (2, 2), (3, 1), (2, 3),
                (3, 2), (3, 3),
            ]
            for t, q in ORDER:
                emit_quarter(t, q)
            for t in range(NQT, N_TILES - 2):
                load_x(t)
                nc.vector.affine_mul_reduce(
                    out=prod_PK[:],
                    accum_out=acc_PT[:, t : t + 1],
                    in0=x_tiles[t][:],
                    in1=w_PK[:],
                    scale=1.0,
                    bias=0.0,
                )

            # The last two tiles are split (loads AND compute: quarters
            # for t=14, eighths for t=15) so the final compute piece
            # starts on the last 256 KiB rather than the last 2 MiB.
            acc_last = cpool.tile([P, 12], mybir.dt.float32)

            def split_tile(t, nsplit, acc_off):
                seg_k = IN_SIZE // nsplit
                x_PK = xpool.tile([P, IN_SIZE], mybir.dt.float32)
                x_tiles[t] = x_PK
                for s in range(nsplit):
                    seg = slice(s * seg_k, (s + 1) * seg_k)
                    nc.sync.dma_start(out=x_PK[:, seg], in_=xt[t][:, seg])
                    nc.vector.affine_mul_reduce(
                        out=prod_PK[:, seg],
                        accum_out=acc_last[:, acc_off + s : acc_off + s + 1],
                        in0=x_PK[:, seg],
                        in1=w_PK[:, seg],
                        scale=1.0,
                        bias=0.0,
                    )

            t14, t15 = N_TILES - 2, N_TILES - 1
            split_tile(t14, 4, 0)
            nc.vector.reduce_sum(
                out=acc_PT[:, t14 : t14 + 1],
                in_=acc_last[:, 0:4],
                axis=mybir.AxisListType.X,
            )
            split_tile(t15, 8, 4)
            nc.vector.reduce_sum(
                out=acc_PT[:, t15 : t15 + 1],
                in_=acc_last[:, 4:12],
                axis=mybir.AxisListType.X,
            )

            for acc_q in accs_q:
                nc.vector.tensor_add(
                    acc_PT[:, 0:NQT], acc_PT[:, 0:NQT], acc_q[:]
                )

            # Output path: tanh(tanh(acc + b)) on ScalarE first (the
            # DVE->ACT handoff needs no DVE drain, unlike DVE->PE), then
            # TensorE-transpose [128, 16] -> [16, 128] so the output DMA
            # writes 512B-contiguous runs (the partition-major layout cost
            # a 12 us completion wait: 2048 4-byte descriptors).
            y_PT = cpool.tile([P, N_TILES], mybir.dt.float32)
            nc.scalar.activation(
                y_PT[:],
                acc_PT[:],
                mybir.ActivationFunctionType.Tanh,
                bias=b_P1[:],
            )
            nc.scalar.activation(y_PT[:], y_PT[:], mybir.ActivationFunctionType.Tanh)
            y_psum = ppool.tile([N_TILES, P], mybir.dt.float32)
            nc.tensor.transpose(y_psum[:], y_PT[:], ident[:])
            # Issue the output DMA from the scalar ring: ScalarE just wrote
            # y_TP, so this skips the ScalarE->Sync semaphore hop at the
            # kernel end, and the sync sequencer is still busy with x-load
            # completions at that point.
            y_TP = cpool.tile([N_TILES, P], mybir.dt.float32)
            nc.scalar.copy(y_TP[:], y_psum[:])
            nc.scalar.dma_start(out=yT, in_=y_TP[:])
    nc.compile()
    return nc


def _get_nc():
    global _NC_CACHE
    if _NC_CACHE is None:
        _NC_CACHE = _build()
    return _NC_CACHE


def _run(x, w, b, **spmd_kwargs):
    """Shard, execute on 8 cores, gather. Returns (out, BassKernelResults)."""
    x = np.ascontiguousarray(np.asarray(x, dtype=np.float32))
    w = np.ascontiguousarray(np.asarray(w, dtype=np.float32))
    b = np.ascontiguousarray(np.asarray(b, dtype=np.float32))
    assert x.shape == (BATCH, IN_SIZE), x.shape

    nc = _get_nc()
    in_maps = [
        {"x": x[c * B_PER_CORE : (c + 1) * B_PER_CORE], "w": w, "b": b}
        for c in range(N_CORES)
    ]
    res = run_bass_kernel_spmd(nc, in_maps, list(range(N_CORES)), **spmd_kwargs)
    out = np.concatenate(
        [np.asarray(res.results[c]["y"]) for c in range(N_CORES)], axis=0
    )
    return out.astype(np.float32, copy=False), res


def kernel(x, w, b):
    try:
        out, _ = _run(x, w, b)
    except Exception:
        # Transient device-wedge (NRT_EXEC_UNIT_UNRECOVERABLE) has been
        # observed once on a first run and succeeded on retry.
        out, _ = _run(x, w, b)
    return out
